# revision 2
# baseline (speedup 1.0000x reference)
"""Chamfer distance (dist1 mean only) on 8 trn2 NeuronCores — v3.

Data-parallel over batch B=8 (one batch per core, host sums partials).

Host index (per batch): Morton-sort both clouds; x into 64 chunks of 128,
y into 1024 blocks of 8 (centroid/radius/representative).  Ball-bound
pruning keeps, per x-chunk, only y-blocks that can contain some chunk
point's nearest neighbor — provably complete, so the device result is
exact.  Chunks are processed in descending candidate-count order so all 8
cores share one BIR program (slot s padded to the rank-s max size).

Device: one K=13 fp16 matmul per slot piece computes psum = -d/2 directly:
  xh·yh + xh·yl + xl·yh - 0.5(y2h+y2l) + (a+b)·1,  a+b = -x2/2 (fp16 split)
(two-term fp16 splits recover fp32-level accuracy at 1 col/cycle).  Slots
are packed into shared 2048-element PSUM tiles; each tile is drained either
by VectorE MAX straight from PSUM (1x) or by ScalarE downcast to fp16 SBUF
followed by VectorE MAX at the 16-bit rate — the two paths run concurrently
on different tiles.  All slots land in one fp16 Mneg row [128, 64]; the
epilogue sums -2*Mneg and partition-reduces with a ones-matmul.
"""

from contextlib import ExitStack

import numpy as np

import concourse.bass as bass
import concourse.tile as tile
from concourse import bacc
from concourse import mybir
from concourse.bass_utils import run_bass_kernel_spmd

F32 = mybir.dt.float32
F16 = mybir.dt.float16

B = 8
PTS = 8192
P = 128
NCHUNK = PTS // P       # 64
G = 8                   # y-block granularity
NBLK = PTS // G
PAD = 128               # slot size granularity
TILE = 2048             # PSUM tile elems (4 banks)
SCALE = 1.0 / (B * PTS)
BFRAC = 0.72            # fraction of elems drained via ScalarE downcast


# ------------------------------------------------------------------ host index

def _morton3(p):
    lo = p.min(0)
    hi = p.max(0)
    q = ((p - lo) / (hi - lo + 1e-9) * 1023.0).astype(np.uint32)

    def spread(v):
        v = v.astype(np.uint64)
        v = (v | (v << np.uint64(16))) & np.uint64(0x030000FF)
        v = (v | (v << np.uint64(8))) & np.uint64(0x0300F00F)
        v = (v | (v << np.uint64(4))) & np.uint64(0x030C30C3)
        v = (v | (v << np.uint64(2))) & np.uint64(0x09249249)
        return v

    return spread(q[:, 0]) | (spread(q[:, 1]) << np.uint64(1)) | (
        spread(q[:, 2]) << np.uint64(2)
    )


def _cand_lists(x, y):
    xs = x[np.argsort(_morton3(x), kind="stable")]
    ys = y[np.argsort(_morton3(y), kind="stable")]
    yb = ys.reshape(NBLK, G, 3)
    cY = yb.mean(1)
    rY = np.sqrt(((yb - cY[:, None]) ** 2).sum(-1)).max(1)
    k = np.argmin(((yb - cY[:, None]) ** 2).sum(-1), axis=1)
    rep = yb[np.arange(NBLK), k]

    x2 = (xs * xs).sum(1)
    rep2 = (rep * rep).sum(1)
    cY2 = (cY * cY).sum(1)
    d_rep = np.sqrt(np.maximum(x2[:, None] + rep2[None] - 2.0 * (xs @ rep.T), 0.0))
    d_cen = np.sqrt(np.maximum(x2[:, None] + cY2[None] - 2.0 * (xs @ cY.T), 0.0))
    ub = d_rep.min(1)
    eps = 1e-3
    lb = d_cen - rY[None]
    cand = lb <= (ub[:, None] + eps)
    lists = []
    for c in range(NCHUNK):
        m = cand[c * P : (c + 1) * P].any(0)
        lists.append(np.flatnonzero(m))
    order = np.argsort([-len(l) for l in lists], kind="stable")
    return xs, ys, [lists[c] for c in order], order


def _split16(v):
    vh = v.astype(np.float16)
    vl = (v - vh.astype(np.float32)).astype(np.float16)
    return vh, vl


def _make_arrays(xs, ys, lists, order, slot_sizes):
    xh, xl = _split16(xs)
    y2 = (ys * ys).sum(1)
    y2h, y2l = _split16(y2)
    yh, yl = _split16(ys)
    x2 = (xs * xs).sum(1)
    nx2 = -0.5 * x2
    a, bb = _split16(nx2)

    lhsT = np.empty((13, PTS), np.float16)
    for s, c in enumerate(order):
        sl = slice(c * P, (c + 1) * P)
        col = slice(s * P, (s + 1) * P)
        lhsT[0:3, col] = xh[sl].T
        lhsT[3:6, col] = xh[sl].T
        lhsT[6:9, col] = xl[sl].T
        lhsT[11, col] = a[sl]
        lhsT[12, col] = bb[sl]
    lhsT[9:11] = np.float16(-0.5)

    tot = sum(slot_sizes)
    rhs = np.empty((13, tot), np.float16)
    rhs[11:13] = np.float16(1.0)
    off = 0
    for s, ids in enumerate(lists):
        sz = slot_sizes[s]
        idsp = np.resize(ids, sz // G)
        pts = (idsp[:, None] * G + np.arange(G)[None, :]).reshape(-1)
        seg = slice(off, off + sz)
        rhs[0:3, seg] = yh[pts].T
        rhs[3:6, seg] = yl[pts].T
        rhs[6:9, seg] = yh[pts].T
        rhs[9, seg] = y2h[pts]
        rhs[10, seg] = y2l[pts]
        off += sz
    return {"lhsT": lhsT, "rhs": rhs}


# ------------------------------------------------------------------ device

def _plan_tiles(slot_sizes):
    """Pack slots (in order) into PSUM tiles of <= TILE elems.
    Returns (tiles, bigs): tiles = [(first_slot, [sizes])] for normal slots,
    bigs = [(slot, [piece sizes])] for slots > TILE (dedicated tiles per
    piece, combined via a scratch row)."""
    tiles = []
    bigs = []
    cur_start, cur = 0, []
    acc = 0
    for s, sz in enumerate(slot_sizes):
        if sz > TILE:
            if cur:
                tiles.append((cur_start, cur))
                cur, acc = [], 0
            pieces = []
            rem = sz
            while rem > 0:
                pieces.append(min(TILE, rem))
                rem -= pieces[-1]
            bigs.append((s, pieces))
            continue
        if acc + sz > TILE and cur:
            tiles.append((cur_start, cur))
            cur, acc = [], 0
        if not cur:
            cur_start = s
        cur.append(sz)
        acc += sz
    if cur:
        tiles.append((cur_start, cur))
    return tiles, bigs


def build(slot_sizes):
    tot = sum(slot_sizes)
    nc = bacc.Bacc(None)
    lhsT_d = nc.declare_dram_parameter("lhsT", [13, PTS], F16, isOutput=False)
    rhs_d = nc.declare_dram_parameter("rhs", [13, tot], F16, isOutput=False)
    out = nc.declare_dram_parameter("out", [1, 1], F32, isOutput=True)

    tiles, bigs = _plan_tiles(slot_sizes)
    # per-tile drain path: True -> ScalarE downcast + fp16 MAX (B), else
    # direct PSUM MAX (A).  Greedy to put ~BFRAC of elems on B.
    b_elems = 0.0
    tile_path = []
    done = 0.0
    for _, sizes in tiles:
        e = float(sum(sizes))
        tile_path.append(b_elems < BFRAC * (done + e))
        if tile_path[-1]:
            b_elems += e
        done += e

    col_off = [0]
    for sz in slot_sizes:
        col_off.append(col_off[-1] + sz)

    with ExitStack() as ctx:
        tc = ctx.enter_context(tile.TileContext(nc))
        singles = ctx.enter_context(tc.tile_pool(name="singles", bufs=1))
        ps_pool = ctx.enter_context(tc.tile_pool(name="ps", bufs=2, space="PSUM"))
        dpool = ctx.enter_context(tc.tile_pool(name="dsl", bufs=3))

        lhsT_sb = singles.tile([13, PTS], F16)
        rhs_sb = singles.tile([13, tot], F16)
        Mneg = singles.tile([128, NCHUNK], F16)
        Mcols = singles.tile([128, 16], F16)
        scr = singles.tile([128, NCHUNK], F32)

        # DMAs: alternate sync/scalar queues; rhs in ~6 col segments.
        nseg = 6
        seg = [int(round(i * tot / nseg)) for i in range(nseg + 1)]
        qs = [nc.sync, nc.scalar]
        qi = 0
        for i in range(nseg):
            eng = qs[qi % 2]
            qi += 1
            eng.dma_start(
                out=rhs_sb[:, seg[i] : seg[i + 1]],
                in_=rhs_d[:, seg[i] : seg[i + 1]],
            )
            if i == 0:
                eng2 = qs[qi % 2]
                qi += 1
                eng2.dma_start(
                    out=lhsT_sb[:, 0 : 16 * P], in_=lhsT_d[:, 0 : 16 * P]
                )
            if i == 1:
                eng2 = qs[qi % 2]
                qi += 1
                eng2.dma_start(
                    out=lhsT_sb[:, 16 * P :], in_=lhsT_d[:, 16 * P :]
                )

        def emit_matmuls(ps, s, coff, lo, ln):
            """slot s candidates [lo, lo+ln) -> ps[:, coff:coff+ln),
            split at 512 (PSUM bank) boundaries."""
            a0 = coff
            while a0 < coff + ln:
                a1 = min((a0 // 512 + 1) * 512, coff + ln)
                nc.tensor.matmul(
                    out=ps[:, a0:a1],
                    lhsT=lhsT_sb[:, s * P : (s + 1) * P],
                    rhs=rhs_sb[
                        :, col_off[s] + lo + (a0 - coff) : col_off[s] + lo + (a1 - coff)
                    ],
                    start=True,
                    stop=True,
                )
                a0 = a1

        def drain_src(ps, telems, via_b):
            if via_b:
                dsl = dpool.tile([128, TILE], F16, tag="dsl")
                nc.scalar.activation(
                    out=dsl[:, 0:telems],
                    in_=ps[:, 0:telems],
                    func=mybir.ActivationFunctionType.Copy,
                    bias=0.0,
                    scale=1.0,
                )
                return dsl
            return ps

        for (s0, sizes), via_b in zip(tiles, tile_path):
            telems = sum(sizes)
            ps = ps_pool.tile([128, TILE], F32, tag="ps")
            toff = 0
            for j, sz in enumerate(sizes):
                emit_matmuls(ps, s0 + j, toff, 0, sz)
                toff += sz
            src = drain_src(ps, telems, via_b)
            # drain runs of equal-size slots with one MAX each
            j = 0
            toff = 0
            while j < len(sizes):
                k = j
                while k < len(sizes) and sizes[k] == sizes[j]:
                    k += 1
                ns = k - j
                run = src[:, toff : toff + ns * sizes[j]].rearrange(
                    "p (s e) -> p s e", s=ns
                )
                nc.vector.tensor_reduce(
                    out=Mneg[:, s0 + j : s0 + k],
                    in_=run,
                    axis=mybir.AxisListType.X,
                    op=mybir.AluOpType.max,
                )
                toff += ns * sizes[j]
                j = k

        # oversized slots: one tile per piece, combine via Mcols scratch
        for s, pieces in bigs:
            lo = 0
            for pidx, ln in enumerate(pieces):
                ps = ps_pool.tile([128, TILE], F32, tag="ps")
                emit_matmuls(ps, s, 0, lo, ln)
                src = drain_src(ps, ln, True)
                nc.vector.tensor_reduce(
                    out=Mcols[:, pidx : pidx + 1],
                    in_=src[:, 0:ln],
                    axis=mybir.AxisListType.X,
                    op=mybir.AluOpType.max,
                )
                lo += ln
            nc.vector.tensor_reduce(
                out=Mneg[:, s : s + 1],
                in_=Mcols[:, 0 : len(pieces)],
                axis=mybir.AxisListType.X,
                op=mybir.AluOpType.max,
            )

        # epilogue: part[p] = sum_s (-2 * SCALE) * Mneg[p, s]
        part = singles.tile([128, 1], F32)
        nc.vector.tensor_scalar(
            out=scr,
            in0=Mneg,
            scalar1=-2.0 * SCALE,
            scalar2=None,
            op0=mybir.AluOpType.mult,
            op1=mybir.AluOpType.add,
            accum_out=part,
        )
        ones_col = singles.tile([128, 1], F32)
        nc.vector.memset(ones_col, 1.0)
        ps_fin = ps_pool.tile([1, 1], F32, tag="ps")
        nc.tensor.matmul(out=ps_fin, lhsT=part, rhs=ones_col, start=True, stop=True)
        out_sb = singles.tile([1, 1], F32)
        nc.scalar.copy(out=out_sb, in_=ps_fin)
        nc.sync.dma_start(out=out[:], in_=out_sb)

    nc.compile()
    if not nc.is_finalized():
        nc.finalize()
    return nc


# ------------------------------------------------------------------ driver

def _prep(xyz1, xyz2):
    batches = []
    for b in range(B):
        x = np.ascontiguousarray(xyz1[b], dtype=np.float32)
        y = np.ascontiguousarray(xyz2[b], dtype=np.float32)
        batches.append(_cand_lists(x, y))
    slot_sizes = []
    for s in range(NCHUNK):
        m = max(len(bt[2][s]) * G for bt in batches)
        slot_sizes.append(max(PAD, ((m + PAD - 1) // PAD) * PAD))
    in_maps = []
    for xs, ys, lists, order in batches:
        in_maps.append(_make_arrays(xs, ys, lists, order, slot_sizes))
    return slot_sizes, in_maps


def _run(xyz1, xyz2, trace=False):
    slot_sizes, in_maps = _prep(np.asarray(xyz1), np.asarray(xyz2))
    nc = build(slot_sizes)
    res = run_bass_kernel_spmd(nc, in_maps, list(range(B)), trace=trace)
    total = np.float64(0.0)
    for r in res.results:
        total += np.float64(r["out"][0, 0])
    return np.asarray(total, dtype=np.float32), res


def kernel(xyz1, xyz2):
    out, _ = _run(np.asarray(xyz1), np.asarray(xyz2), trace=False)
    return out
